# Initial kernel scaffold
#
"""Trainium2 Bass kernel for nn_Attention_63127429317226.

out[d] = sum_t softmax_d(W * r_star * q_t)[t, d] * q_t[t, d]
  T = 32768, D = 1024.  (The scalar bias b is softmax-invariant and drops out.)

Strategy: shard T across 8 cores (4096 rows each). Per [128, 1024] tile:
  beta = q * (W*r_star)          (DVE tensor_tensor)
  e    = exp(beta), s = row-sum  (ACT, fused accum_out)
  r    = 1/s                     (DVE reciprocal)
  qn   = q * r                   (DVE per-partition tensor_scalar)
  acc[b] += e[:,b]^T @ qn[:,b]   (PE, 8 accumulating matmuls; only the
                                  diagonal of each block is the answer — the
                                  PE computes the e*q products + t-reduction)
Epilogue: diag extract via identity mask-mul + segmented reduce -> [128, 8]
partial sums per core; host adds the 8 cores' partials and reorders to [1024].

Two precision/speed modes:
  "f32r": f32 datapath, float32r matmuls with 256-wide moving operand
          (full PE rate).  ~2e-4 scale-relative absmax.
  "fp16": q converted to fp16 host-side (halves HBM traffic), fp16 DVE
          fast modes (tensor_tensor 2x, tensor_scalar 4x) and fp16 matmuls.
"""

import os
import sys
from contextlib import ExitStack

import numpy as np

for _p in ("/opt/trn_rl_repo", "/root/.axon_site/_ro/trn_rl_repo"):
    if os.path.isdir(_p) and _p not in sys.path:
        sys.path.insert(0, _p)

import concourse.bacc as bacc
import concourse.tile as tile
from concourse import mybir
from concourse.bass_utils import run_bass_kernel_spmd

D = 1024
T = 32768
N_CORES = 8
P = 128
N_BLK = D // P  # 8

F32 = mybir.dt.float32
F32R = mybir.dt.float32r
FP16 = mybir.dt.float16

MODE = os.environ.get("KERNEL_MODE", "fp16")


def _n_mm(mode: str) -> int:
    # f32r needs a >=256-wide moving operand for full PE rate; fp16 doesn't.
    return 256 if mode == "f32r" else P


def _rhs_start(b: int, mode: str) -> int:
    return min(b * P, D - _n_mm(mode))


def build_nc(t_shard: int, mode: str = MODE):
    """Build the single-core Bass program for a T-shard of `t_shard` rows."""
    assert t_shard % P == 0
    n_tiles = t_shard // P
    n_mm = _n_mm(mode)
    dt_q = FP16 if mode == "fp16" else F32
    dt_mm = FP16 if mode == "fp16" else F32R

    nc = bacc.Bacc(None)
    q = nc.dram_tensor("q", [t_shard, D], dt_q, kind="ExternalInput")
    # scale = W * r_star pre-broadcast to [128, D] on host (pure input prep)
    scale = nc.dram_tensor("scale", [P, D], dt_q, kind="ExternalInput")
    eye = nc.dram_tensor("eye", [P, N_BLK * n_mm], dt_q, kind="ExternalInput")
    out = nc.dram_tensor("out", [P, N_BLK], F32, kind="ExternalOutput")

    import types as _types

    from concourse.vector_clock import ScopedClock as _ScopedClock

    def _minimal_drain(self, tick_clock, wait_clock):
        # Slim kernel exit: keep the completion-join drain (Sync waits for
        # every proc's final tick, so the NEFF completes only when all work
        # is done) but skip the two all-engine barriers + sem-clear
        # instructions — the Bass preamble re-clears the sem range at the
        # start of every execution, so exit-time clears are redundant for
        # re-execution.  Saves several us of kernel-tail barrier time.
        drain_inst = self.nc.sync.drain()
        wait_clock.add_sem_waits(
            drain_inst.ins, _ScopedClock({None: tick_clock.global_clock})
        )
        popped = self.nc._tile_sem_poison_stack.pop()
        assert popped is self._sem_poison

    with tile.TileContext(nc) as tc, ExitStack() as ctx:
        if os.environ.get("KERNEL_FASTEXIT", "1") == "1":
            tc._drain_and_barrier = _types.MethodType(_minimal_drain, tc)
        singles = ctx.enter_context(tc.tile_pool(name="singles", bufs=1))
        qpool = ctx.enter_context(tc.tile_pool(name="qpool", bufs=20))
        bpool = ctx.enter_context(tc.tile_pool(name="bpool", bufs=8))
        epool = ctx.enter_context(tc.tile_pool(name="epool", bufs=8))
        npool = ctx.enter_context(tc.tile_pool(name="npool", bufs=8))
        spool = ctx.enter_context(tc.tile_pool(name="spool", bufs=12))
        psum = ctx.enter_context(tc.tile_pool(name="psum", bufs=1, space="PSUM"))

        # one full 2KB PSUM bank per accumulation chain (zero-region granularity)
        acc = psum.tile([P, N_BLK, 512], F32)

        scale_b = singles.tile([P, D], dt_q)
        nc.sync.dma_start(out=scale_b, in_=scale[:])

        # Work items: (row0, nrows, start_flag).  With deep prefetch
        # buffers plain full tiles beat sub-chunking the first tile — every
        # extra chunk costs a full-overhead ACTIVATE on the ACT-bound path.
        items = [(i * P, P, i == 0) for i in range(n_tiles)]

        def emit_front(it):
            row0, nr, _ = it
            qt = qpool.tile([P, D], dt_q, name="qt")
            nc.sync.dma_start(out=qt[:nr, :], in_=q[row0 : row0 + nr, :])
            beta = bpool.tile([P, D], dt_q, name="beta")
            nc.vector.tensor_mul(beta[:nr, :], qt[:nr, :], scale_b[:nr, :])
            e = epool.tile([P, D], dt_mm, name="e")
            s = spool.tile([P, 1], F32, name="s")
            nc.scalar.activation(
                e[:nr, :],
                beta[:nr, :],
                mybir.ActivationFunctionType.Exp,
                accum_out=s[:nr, :],
            )
            return qt, e, s

        def emit_back(it, fr, last):
            row0, nr, start = it
            qt, e, s = fr
            r = spool.tile([P, 1], F32, name="r")
            nc.vector.reciprocal(r[:nr, :], s[:nr, :])
            qn = npool.tile([P, D], dt_mm, name="qn")
            nc.vector.tensor_scalar_mul(qn[:nr, :], qt[:nr, :], r[:nr, :])
            for b in range(N_BLK):
                rs = _rhs_start(b, mode)
                nc.tensor.matmul(
                    acc[:, b, :n_mm],
                    e[:nr, b * P : (b + 1) * P],
                    qn[:nr, rs : rs + n_mm],
                    start=start,
                    stop=last,
                )

        for idx, it in enumerate(items):
            fr = emit_front(it)
            emit_back(it, fr, last=(idx == len(items) - 1))

        # --- epilogue: extract the 8 block diagonals -> [P, N_BLK] ---
        # (eye load emitted last so its DMA never delays the q stream;
        # two block-halves pipeline mul/reduce/DMA-out)
        eye_sb = singles.tile([P, N_BLK, n_mm], dt_q)
        nc.sync.dma_start(
            out=eye_sb, in_=eye[:].rearrange("p (b j) -> p b j", j=n_mm)
        )
        masked = singles.tile([P, N_BLK, n_mm], F32)
        dout = singles.tile([P, N_BLK], F32)
        h = N_BLK // 2
        for k in range(2):
            blks = slice(k * h, (k + 1) * h)
            nc.vector.tensor_mul(
                masked[:, blks, :], acc[:, blks, :n_mm], eye_sb[:, blks, :]
            )
            nc.vector.tensor_reduce(
                dout[:, blks],
                masked[:, blks, :],
                axis=mybir.AxisListType.X,
                op=mybir.AluOpType.add,
            )
            nc.sync.dma_start(out=out[:, blks], in_=dout[:, blks])

    nc.compile()
    return nc


_NC_CACHE: dict = {}


def _get_nc(t_shard: int, mode: str = MODE):
    key = (t_shard, mode)
    if key not in _NC_CACHE:
        _NC_CACHE[key] = build_nc(t_shard, mode)
    return _NC_CACHE[key]


def _make_eye(mode: str = MODE) -> np.ndarray:
    # eye[p, b*n_mm + (b*P - rhs_start(b)) + p] = 1 -> picks block b's diagonal
    n_mm = _n_mm(mode)
    dt = np.float16 if mode == "fp16" else np.float32
    eye = np.zeros((P, N_BLK * n_mm), dtype=dt)
    for b in range(N_BLK):
        off = b * P - _rhs_start(b, mode)
        eye[np.arange(P), b * n_mm + off + np.arange(P)] = 1.0
    return eye


def _make_scale(w: np.ndarray, r_star: np.ndarray, mode: str = MODE) -> np.ndarray:
    dt = np.float16 if mode == "fp16" else np.float32
    return np.ascontiguousarray(
        np.broadcast_to((w * r_star)[None, :].astype(dt), (P, D))
    )


def kernel(**inputs) -> np.ndarray:
    q_t = np.ascontiguousarray(np.asarray(inputs["q_t"], dtype=np.float32))
    r_star = np.asarray(inputs["r_star"], dtype=np.float32)
    w = np.asarray(inputs["W"], dtype=np.float32)
    # inputs["b"] is a scalar bias added uniformly before a softmax over d:
    # softmax(x + c) == softmax(x), so it cannot affect the output.

    t_total = q_t.shape[0]
    t_shard = t_total // N_CORES
    nc = _get_nc(t_shard)
    eye = _make_eye()
    scale = _make_scale(w, r_star)

    if MODE == "fp16":
        q_t = q_t.astype(np.float16)
    shards = q_t.reshape(N_CORES, t_shard, D)
    in_maps = [
        {"q": shards[c], "scale": scale, "eye": eye} for c in range(N_CORES)
    ]
    res = run_bass_kernel_spmd(nc, in_maps, core_ids=list(range(N_CORES)))
    parts = np.stack([res.results[c]["out"] for c in range(N_CORES)])  # [8,128,8]
    total = parts.astype(np.float64).sum(axis=0)  # [128, 8]
    # out[b*128 + p] = total[p, b]
    return np.ascontiguousarray(total.T.reshape(-1)).astype(np.float32)



# revision 1
# speedup vs baseline: 1.5243x; 1.5243x over previous
"""Trainium2 Bass kernel for nn_Attention_63127429317226.

out[d] = sum_t softmax_d(W * r_star * q_t)[t, d] * q_t[t, d]
  T = 32768, D = 1024.  (The scalar bias b is softmax-invariant and drops out.)

Strategy: shard T across 8 cores (4096 rows each). Per [128, 1024] tile:
  beta = q * (W*r_star)          (DVE tensor_tensor)
  e    = exp(beta), s = row-sum  (ACT, fused accum_out)
  r    = 1/s                     (DVE reciprocal)
  qn   = q * r                   (DVE per-partition tensor_scalar)
  acc[b] += e[:,b]^T @ qn[:,b]   (PE, 8 accumulating matmuls; only the
                                  diagonal of each block is the answer — the
                                  PE computes the e*q products + t-reduction)
Epilogue: diag extract via identity mask-mul + segmented reduce -> [128, 8]
partial sums per core; host adds the 8 cores' partials and reorders to [1024].

Two precision/speed modes:
  "f32r": f32 datapath, float32r matmuls with 256-wide moving operand
          (full PE rate).  ~2e-4 scale-relative absmax.
  "fp16": q converted to fp16 host-side (halves HBM traffic), fp16 DVE
          fast modes (tensor_tensor 2x, tensor_scalar 4x) and fp16 matmuls.
"""

import os
import sys
from contextlib import ExitStack

import numpy as np

for _p in ("/opt/trn_rl_repo", "/root/.axon_site/_ro/trn_rl_repo"):
    if os.path.isdir(_p) and _p not in sys.path:
        sys.path.insert(0, _p)

import concourse.bacc as bacc
import concourse.tile as tile
from concourse import mybir
from concourse.bass_utils import run_bass_kernel_spmd

D = 1024
T = 32768
N_CORES = 8
P = 128
N_BLK = D // P  # 8

F32 = mybir.dt.float32
F32R = mybir.dt.float32r
FP16 = mybir.dt.float16

MODE = os.environ.get("KERNEL_MODE", "fp16")


def _n_mm(mode: str) -> int:
    # f32r needs a >=256-wide moving operand for full PE rate; fp16 doesn't.
    return 256 if mode == "f32r" else P


def _rhs_start(b: int, mode: str) -> int:
    return min(b * P, D - _n_mm(mode))


def build_nc(t_shard: int, mode: str = MODE):
    """Build the single-core Bass program for a T-shard of `t_shard` rows."""
    assert t_shard % P == 0
    n_tiles = t_shard // P
    n_mm = _n_mm(mode)
    dt_q = FP16 if mode == "fp16" else F32
    dt_mm = FP16 if mode == "fp16" else F32R

    nc = bacc.Bacc(None)
    q = nc.dram_tensor("q", [t_shard, D], dt_q, kind="ExternalInput")
    # scale = W * r_star pre-broadcast to [128, D] on host (pure input prep)
    scale = nc.dram_tensor("scale", [P, D], dt_q, kind="ExternalInput")
    eye = nc.dram_tensor("eye", [P, N_BLK * n_mm], dt_q, kind="ExternalInput")
    out = nc.dram_tensor("out", [P, N_BLK], F32, kind="ExternalOutput")

    import types as _types

    from concourse.vector_clock import ScopedClock as _ScopedClock

    def _minimal_drain(self, tick_clock, wait_clock):
        # Slim kernel exit: keep the completion-join drain (Sync waits for
        # every proc's final tick, so the NEFF completes only when all work
        # is done) but skip the two all-engine barriers + sem-clear
        # instructions — the Bass preamble re-clears the sem range at the
        # start of every execution, so exit-time clears are redundant for
        # re-execution.  Saves several us of kernel-tail barrier time.
        drain_inst = self.nc.sync.drain()
        wait_clock.add_sem_waits(
            drain_inst.ins, _ScopedClock({None: tick_clock.global_clock})
        )
        popped = self.nc._tile_sem_poison_stack.pop()
        assert popped is self._sem_poison

    with tile.TileContext(nc) as tc, ExitStack() as ctx:
        if os.environ.get("KERNEL_FASTEXIT", "1") == "1":
            tc._drain_and_barrier = _types.MethodType(_minimal_drain, tc)
        singles = ctx.enter_context(tc.tile_pool(name="singles", bufs=1))
        qpool = ctx.enter_context(tc.tile_pool(name="qpool", bufs=20))
        bpool = ctx.enter_context(tc.tile_pool(name="bpool", bufs=8))
        epool = ctx.enter_context(tc.tile_pool(name="epool", bufs=8))
        npool = ctx.enter_context(tc.tile_pool(name="npool", bufs=8))
        spool = ctx.enter_context(tc.tile_pool(name="spool", bufs=12))
        psum = ctx.enter_context(tc.tile_pool(name="psum", bufs=1, space="PSUM"))

        # one full 2KB PSUM bank per accumulation chain (zero-region granularity)
        acc = psum.tile([P, N_BLK, 512], F32)

        scale_b = singles.tile([P, D], dt_q)
        nc.sync.dma_start(out=scale_b, in_=scale[:])

        # Work items: (row0, nrows, start_flag).  With deep prefetch
        # buffers plain full tiles beat sub-chunking the first tile — every
        # extra chunk costs a full-overhead ACTIVATE on the ACT-bound path.
        items = [(i * P, P, i == 0) for i in range(n_tiles)]

        def emit_front(it):
            row0, nr, _ = it
            qt = qpool.tile([P, D], dt_q, name="qt")
            nc.sync.dma_start(out=qt[:nr, :], in_=q[row0 : row0 + nr, :])
            beta = bpool.tile([P, D], dt_q, name="beta")
            nc.vector.tensor_mul(beta[:nr, :], qt[:nr, :], scale_b[:nr, :])
            e = epool.tile([P, D], dt_mm, name="e")
            s = spool.tile([P, 1], F32, name="s")
            nc.scalar.activation(
                e[:nr, :],
                beta[:nr, :],
                mybir.ActivationFunctionType.Exp,
                accum_out=s[:nr, :],
            )
            return qt, e, s

        def emit_back(it, fr, last):
            row0, nr, start = it
            qt, e, s = fr
            r = spool.tile([P, 1], F32, name="r")
            nc.vector.reciprocal(r[:nr, :], s[:nr, :])
            qn = npool.tile([P, D], dt_mm, name="qn")
            nc.vector.tensor_scalar_mul(qn[:nr, :], qt[:nr, :], r[:nr, :])
            for b in range(N_BLK):
                rs = _rhs_start(b, mode)
                nc.tensor.matmul(
                    acc[:, b, :n_mm],
                    e[:nr, b * P : (b + 1) * P],
                    qn[:nr, rs : rs + n_mm],
                    start=start,
                    stop=last,
                )

        for idx, it in enumerate(items):
            fr = emit_front(it)
            emit_back(it, fr, last=(idx == len(items) - 1))

        # --- epilogue: extract the 8 block diagonals -> [P, N_BLK] ---
        # (eye load emitted last so its DMA never delays the q stream;
        # two block-halves pipeline mul/reduce/DMA-out)
        eye_sb = singles.tile([P, N_BLK, n_mm], dt_q)
        nc.sync.dma_start(
            out=eye_sb, in_=eye[:].rearrange("p (b j) -> p b j", j=n_mm)
        )
        masked = singles.tile([P, N_BLK, n_mm], F32)
        dout = singles.tile([P, N_BLK], F32)
        h = N_BLK // 2
        for k in range(2):
            blks = slice(k * h, (k + 1) * h)
            nc.vector.tensor_mul(
                masked[:, blks, :], acc[:, blks, :n_mm], eye_sb[:, blks, :]
            )
            nc.vector.tensor_reduce(
                dout[:, blks],
                masked[:, blks, :],
                axis=mybir.AxisListType.X,
                op=mybir.AluOpType.add,
            )
            nc.sync.dma_start(out=out[:, blks], in_=dout[:, blks])

    nc.compile()
    return nc


_NC_CACHE: dict = {}


def _get_nc(t_shard: int, mode: str = MODE):
    key = (t_shard, mode)
    if key not in _NC_CACHE:
        _NC_CACHE[key] = build_nc(t_shard, mode)
    return _NC_CACHE[key]


def _make_eye(mode: str = MODE) -> np.ndarray:
    # eye[p, b*n_mm + (b*P - rhs_start(b)) + p] = 1 -> picks block b's diagonal
    n_mm = _n_mm(mode)
    dt = np.float16 if mode == "fp16" else np.float32
    eye = np.zeros((P, N_BLK * n_mm), dtype=dt)
    for b in range(N_BLK):
        off = b * P - _rhs_start(b, mode)
        eye[np.arange(P), b * n_mm + off + np.arange(P)] = 1.0
    return eye


def _make_scale(w: np.ndarray, r_star: np.ndarray, mode: str = MODE) -> np.ndarray:
    dt = np.float16 if mode == "fp16" else np.float32
    return np.ascontiguousarray(
        np.broadcast_to((w * r_star)[None, :].astype(dt), (P, D))
    )


def kernel(**inputs) -> np.ndarray:
    q_t = np.ascontiguousarray(np.asarray(inputs["q_t"], dtype=np.float32))
    r_star = np.asarray(inputs["r_star"], dtype=np.float32)
    w = np.asarray(inputs["W"], dtype=np.float32)
    # inputs["b"] is a scalar bias added uniformly before a softmax over d:
    # softmax(x + c) == softmax(x), so it cannot affect the output.

    t_total = q_t.shape[0]
    t_shard = t_total // N_CORES
    nc = _get_nc(t_shard)
    eye = _make_eye()
    scale = _make_scale(w, r_star)

    if MODE == "fp16":
        q_t = q_t.astype(np.float16)
    shards = q_t.reshape(N_CORES, t_shard, D)
    in_maps = [
        {"q": shards[c], "scale": scale, "eye": eye} for c in range(N_CORES)
    ]
    res = run_bass_kernel_spmd(nc, in_maps, core_ids=list(range(N_CORES)))
    parts = np.stack([res.results[c]["out"] for c in range(N_CORES)])  # [8,128,8]
    total = parts.astype(np.float64).sum(axis=0)  # [128, 8]
    # out[b*128 + p] = total[p, b]
    return np.ascontiguousarray(total.T.reshape(-1)).astype(np.float32)

